# revision 12
# baseline (speedup 1.0000x reference)
"""Trainium2 Bass kernel: custom inverse STFT (per-bin rotation + Hann window
+ overlap-add + window correction), fp16 device path.

Math (matching the reference):
    F[i,k] = S_real[i,k]*A[k] + S_imag[i,k]*B[k]
      A[k] = w[k]*(cos(th)-sin(th))/n,  B[k] = -w[k]*(cos(th)+sin(th))/n
    out[t] = sum_i F[i, t-256*i] / max(corr[t], 1e-8)

Sharding: 8192 frames -> 8 cores x 1024 frames.  Each core computes a padded
per-partition overlap-add segment; ALL overlaps (across partitions and across
cores) are resolved on the host by strided adds, so the device kernel has no
halo exchange at all.

Per-core layout: partition p holds frames 8p..8p+7 (16KB fp16 contiguous in
DRAM per partition).  Device pipeline per core:
  - inputs stream in fp16 quarters (2 frames/partition each) on the sync queue
  - DVE computes products P1 = Sr*A~, P2 = Si*B~ (A~/B~ broadcast along the
    frame axis with a stride-0 AP); coefficients are pre-scaled by 256 on the
    host so fp16 products stay in the normal range
  - PE overlap-adds: for each 512-f32 PSUM window of the padded block axis,
    8 identity-lhsT matmuls (4 chunk shifts x 2 tensors) accumulate shifted
    reads of P1/P2 into PSUM; P1/P2 carry 3 zeroed pad frames on each side so
    every matmul in a window covers the identical region
  - ACT copies each finished PSUM window to SBUF as fp16 and the output
    streams out on the scalar queue
Host: assemble per-core [128, 11, 256] padded segments with strided adds,
divide by the precomputed window correction (and the 256 coefficient scale).
"""

import numpy as np

import concourse.bass as bass
import concourse.bacc as bacc
import concourse.mybir as mybir
import concourse.tile as tile
from concourse.bass_utils import run_bass_kernel_spmd

F16 = mybir.dt.float16
F32 = mybir.dt.float32
ALU = mybir.AluOpType

P = 128            # SBUF partitions
G = 8              # frames per partition
FL = 1024          # frame length (== fft length)
FS = 256           # frame step
NF = 8192          # total frames
NCORES = 8
FPC = NF // NCORES          # frames owned per core (1024)
OUT_LEN = FS * (NF - 1) + FL
PAD = 1                     # zero pad frames on each side of product tiles
                            # (all-pad chunk shifts are skipped, so only one
                            # neighbor frame on each side is ever read)
PFR = G + 2 * PAD           # padded frames per partition (14)
NB = G + 3                  # output blocks per partition (11)
OLEN = NB * FS              # 2816 samples per partition
SCALE = 2.0 ** 15           # host-side coefficient prescale: keeps the tiny
                            # edge coefficients (|A[1]|~9e-9, amplified 1e5x by
                            # the window correction) in fp16 NORMAL range
NQ = 4                      # input quarters (2 frames/partition each)


def _window32():
    # bit-matches the reference's f32 window computation
    k = np.arange(FL, dtype=np.float32)
    th = np.float32(2.0 * np.pi) * k / np.float32(FL)
    return (np.float32(0.5) - np.float32(0.5) * np.cos(th)).astype(np.float32)


def _coeffs16():
    k = np.arange(FL, dtype=np.float64)
    th = 2.0 * np.pi * k / FL
    w = _window32().astype(np.float64)
    a = w * (np.cos(th) - np.sin(th)) / FL * SCALE
    b = -w * (np.cos(th) + np.sin(th)) / FL * SCALE
    return a.astype(np.float16), b.astype(np.float16)


def _window_correction():
    w = _window32()
    corr = np.zeros(OUT_LEN, dtype=np.float32)
    for j in range(4):
        chunk = w[j * FS:(j + 1) * FS]
        view = corr[j * FS:j * FS + NF * FS].reshape(NF, FS)
        view += chunk[None, :]
    return corr


def build_nc():
    nc = bacc.Bacc(trn_type="TRN2", target_bir_lowering=False, debug=False)
    sr_d = nc.dram_tensor("s_real", [FPC, FL], F16, kind="ExternalInput").ap()
    si_d = nc.dram_tensor("s_imag", [FPC, FL], F16, kind="ExternalInput").ap()
    ca_d = nc.dram_tensor("coef_a", [1, FL], F16, kind="ExternalInput").ap()
    cb_d = nc.dram_tensor("coef_b", [1, FL], F16, kind="ExternalInput").ap()
    id_d = nc.dram_tensor("ident", [P, P], F16, kind="ExternalInput").ap()
    out_d = nc.dram_tensor("out_seg", [P, OLEN], F16, kind="ExternalOutput").ap()

    sr3 = sr_d.rearrange("(p g) k -> p g k", p=P)
    si3 = si_d.rearrange("(p g) k -> p g k", p=P)

    with tile.TileContext(nc) as tc:
        with (
            tc.tile_pool(name="const", bufs=1) as cpool,
            tc.tile_pool(name="main", bufs=1) as mpool,
            tc.tile_pool(name="psum", bufs=1, space="PSUM") as ppool,
        ):
            At = cpool.tile([P, FL], F16, tag="At")
            Bt = cpool.tile([P, FL], F16, tag="Bt")
            It = cpool.tile([P, P], F16, tag="It")
            Srt = mpool.tile([P, G * FL], F16, tag="Sr")
            Sit = mpool.tile([P, G * FL], F16, tag="Si")
            P1t = mpool.tile([P, PFR * FL], F16, tag="P1")
            P2t = mpool.tile([P, PFR * FL], F16, tag="P2")
            # per-chunk output tiles and per-window PSUM tiles: distinct tags
            # so the tile framework doesn't serialize independent windows on
            # false whole-tile WAR hazards
            windows = [(0, 2), (2, 4), (4, 6), (6, 8), (8, 10), (10, 11)]
            Otc = [mpool.tile([P, n * FS], F16, tag=f"Oc{i}", name=f"Oc{i}")
                   for i, n in enumerate((4, 4, 3))]
            Opw = [ppool.tile([P, (b1 - b0) * FS], F32, tag=f"Ops{w}",
                              name=f"Ops{w}")
                   for w, (b0, b1) in enumerate(windows[:5])]

            # coefficient rows: tiny 2KB DMAs first in the sync queue, then
            # broadcast across partitions on the (otherwise idle) GpSimd —
            # keeps 0.5MB of broadcast traffic out of the HBM stream
            Ar = cpool.tile([1, FL], F16, tag="Ar")
            Br = cpool.tile([1, FL], F16, tag="Br")
            nc.sync.dma_start(out=Ar[:, :], in_=ca_d[:, :])
            nc.sync.dma_start(out=Br[:, :], in_=cb_d[:, :])
            nc.gpsimd.partition_broadcast(At[:, :], Ar[:, :])
            nc.gpsimd.partition_broadcast(Bt[:, :], Br[:, :])
            nc.scalar.dma_start(out=It[:, :], in_=id_d[:, :])

            # zero the pad frames once on DVE (it is idle until the first
            # input quarter lands anyway)
            for T in (P1t, P2t):
                nc.vector.memset(T[:, 0:PAD * FL], 0.0)
                nc.vector.memset(T[:, (PAD + G) * FL:], 0.0)

            # input stream on the sync (SP) queue: 2-frame loads for frames
            # 0..5, single-frame loads for the last two so the product tail
            # after the final transfer is as short as possible
            loads = [(0, 2), (2, 2), (4, 2), (6, 1), (7, 1)]
            for f0, n in loads:
                sl = slice(f0 * FL, (f0 + n) * FL)
                nc.sync.dma_start(out=Srt[:, sl], in_=sr3[:, f0:f0 + n, :])
                nc.sync.dma_start(out=Sit[:, sl], in_=si3[:, f0:f0 + n, :])

            P1v = P1t[:, :].rearrange("p (g c j) -> p g c j", g=PFR, c=4)
            P2v = P2t[:, :].rearrange("p (g c j) -> p g c j", g=PFR, c=4)

            def emit_products(f0, n):
                sl = slice(f0 * FL, (f0 + n) * FL)
                psl = slice((PAD + f0) * FL, (PAD + f0 + n) * FL)
                for S, C, T in ((Srt, At, P1t), (Sit, Bt, P2t)):
                    nc.vector.tensor_tensor(
                        out=T[:, psl].rearrange("p (g k) -> p g k", g=n),
                        in0=S[:, sl].rearrange("p (g k) -> p g k", g=n),
                        in1=C[:, None, :].broadcast_to([P, n, FL]),
                        op=ALU.mult,
                    )

            def emit_window(w):
                # PE identity matmuls accumulate the chunk shifts x 2 tensors
                # into this window's private PSUM bank, then ACT copies to the
                # right output chunk as fp16.  Shifts whose whole frame range
                # falls in the pads are skipped (they would add zeros).
                b0, b1 = windows[w]
                Ow = Opw[w]
                Ov = Ow[:, :].rearrange("p (b j) -> p b j", b=b1 - b0)
                seq = [(t, c) for t in (0, 1) for c in range(4)
                       if (b1 - c > 0) and (b0 - c < G)]
                for i, (t, c) in enumerate(seq):
                    src = P1v if t == 0 else P2v
                    nc.tensor.matmul(
                        Ov[:, :, :], It[:, :],
                        src[:, b0 - c + PAD:b1 - c + PAD, c, :],
                        start=(i == 0), stop=(i == len(seq) - 1),
                    )
                chunk, coff = divmod(b0, 4)
                nc.scalar.copy(out=Otc[chunk][:, coff * FS:(coff + b1 - b0) * FS],
                               in_=Ow[:, :])

            def emit_window5_dve():
                # block 10 has exactly one real contribution pair (frame 7,
                # chunk 3): a single DVE add, running while PE drains windows
                # 3-4 — shortens the post-product tail
                nc.vector.tensor_tensor(
                    out=Otc[2][:, 2 * FS:3 * FS],
                    in0=P1v[:, G - 1 + PAD, 3, :],
                    in1=P2v[:, G - 1 + PAD, 3, :], op=ALU.add)

            # pipeline: products gate windows; emit in readiness order
            emit_products(0, 2)
            emit_products(2, 2)
            emit_window(0)          # frames <= 1
            emit_products(4, 2)
            emit_window(1)          # frames <= 3
            emit_products(6, 1)
            emit_window(2)          # frames <= 5
            nc.scalar.dma_start(out=out_d[:, 0:4 * FS], in_=Otc[0][:, :])
            emit_products(7, 1)
            emit_window(3)          # frames <= 7
            emit_window5_dve()
            emit_window(4)
            nc.scalar.dma_start(out=out_d[:, 4 * FS:8 * FS], in_=Otc[1][:, :])
            nc.scalar.dma_start(out=out_d[:, 8 * FS:], in_=Otc[2][:, :])
    nc.compile()
    return nc


_cache = {}


def _get_nc():
    if "nc" not in _cache:
        _cache["nc"] = build_nc()
    return _cache["nc"]


def make_in_maps(S_real, S_imag):
    a16, b16 = _coeffs16()
    sr16 = S_real.astype(np.float16)
    si16 = S_imag.astype(np.float16)
    ident = np.eye(P, dtype=np.float16)
    in_maps = []
    for m in range(NCORES):
        r0 = m * FPC
        in_maps.append({
            "s_real": np.ascontiguousarray(sr16[r0:r0 + FPC]),
            "s_imag": np.ascontiguousarray(si16[r0:r0 + FPC]),
            "coef_a": a16.reshape(1, FL),
            "coef_b": b16.reshape(1, FL),
            "ident": ident,
        })
    return in_maps


def assemble_output(segs):
    # segs: per core [128, 2816] fp16 padded OA partials
    acc = np.zeros((NF + NB - G, FS), dtype=np.float32)   # [8195, 256]
    for m in range(NCORES):
        seg = segs[m].astype(np.float32).reshape(P, NB, FS)
        accm = acc[FPC * m: FPC * m + FPC + NB - G]       # [1027, 256] view
        for b in range(NB):
            accm[b: b + (P - 1) * G + 1: G] += seg[:, b, :]
    if "corr" not in _cache:
        _cache["corr"] = _window_correction()
    corr = _cache["corr"]
    return (acc.reshape(-1) / (np.maximum(corr, np.float32(1e-8))
                               * np.float32(SCALE))).astype(np.float32)


def kernel(S_real, S_imag):
    S_real = np.asarray(S_real, dtype=np.float32)
    S_imag = np.asarray(S_imag, dtype=np.float32)
    in_maps = make_in_maps(S_real, S_imag)
    nc = _get_nc()
    res = run_bass_kernel_spmd(nc, in_maps, list(range(NCORES)))
    segs = [res.results[m]["out_seg"] for m in range(NCORES)]
    return assemble_output(segs)


# revision 14
# speedup vs baseline: 1.0953x; 1.0953x over previous
"""Trainium2 Bass kernel: custom inverse STFT (per-bin rotation + Hann window
+ overlap-add + window correction), fp16 device path.

Math (matching the reference):
    F[i,k] = S_real[i,k]*A[k] + S_imag[i,k]*B[k]
      A[k] = w[k]*(cos(th)-sin(th))/n,  B[k] = -w[k]*(cos(th)+sin(th))/n
    out[t] = sum_i F[i, t-256*i] / max(corr[t], 1e-8)

Sharding: 8192 frames -> 8 cores x 1024 frames.  Each core computes a padded
per-partition overlap-add segment; ALL overlaps (across partitions and across
cores) are resolved on the host by strided adds, so the device kernel has no
halo exchange at all.

Per-core layout: partition p holds frames 8p..8p+7 (16KB fp16 contiguous in
DRAM per partition).  Device pipeline per core:
  - inputs stream in fp16 quarters (2 frames/partition each) on the sync queue
  - DVE computes products P1 = Sr*A~, P2 = Si*B~ (A~/B~ broadcast along the
    frame axis with a stride-0 AP); coefficients are pre-scaled by 256 on the
    host so fp16 products stay in the normal range
  - PE overlap-adds: for each 512-f32 PSUM window of the padded block axis,
    8 identity-lhsT matmuls (4 chunk shifts x 2 tensors) accumulate shifted
    reads of P1/P2 into PSUM; P1/P2 carry 3 zeroed pad frames on each side so
    every matmul in a window covers the identical region
  - ACT copies each finished PSUM window to SBUF as fp16 and the output
    streams out on the scalar queue
Host: assemble per-core [128, 11, 256] padded segments with strided adds,
divide by the precomputed window correction (and the 256 coefficient scale).
"""

import numpy as np

import concourse.bass as bass
import concourse.bacc as bacc
import concourse.mybir as mybir
import concourse.tile as tile
from concourse.bass_utils import run_bass_kernel_spmd

F16 = mybir.dt.float16
F32 = mybir.dt.float32
ALU = mybir.AluOpType

P = 128            # SBUF partitions
G = 8              # frames per partition
FL = 1024          # frame length (== fft length)
FS = 256           # frame step
NF = 8192          # total frames
NCORES = 8
FPC = NF // NCORES          # frames owned per core (1024)
OUT_LEN = FS * (NF - 1) + FL
PAD = 1                     # zero pad frames on each side of product tiles
                            # (all-pad chunk shifts are skipped, so only one
                            # neighbor frame on each side is ever read)
PFR = G + 2 * PAD           # padded frames per partition (14)
NB = G + 3                  # output blocks per partition (11)
OLEN = NB * FS              # 2816 samples per partition
SCALE = 2.0 ** 15           # host-side coefficient prescale: keeps the tiny
                            # edge coefficients (|A[1]|~9e-9, amplified 1e5x by
                            # the window correction) in fp16 NORMAL range
NQ = 4                      # input quarters (2 frames/partition each)


def _window32():
    # bit-matches the reference's f32 window computation
    k = np.arange(FL, dtype=np.float32)
    th = np.float32(2.0 * np.pi) * k / np.float32(FL)
    return (np.float32(0.5) - np.float32(0.5) * np.cos(th)).astype(np.float32)


def _coeffs16():
    k = np.arange(FL, dtype=np.float64)
    th = 2.0 * np.pi * k / FL
    w = _window32().astype(np.float64)
    a = w * (np.cos(th) - np.sin(th)) / FL * SCALE
    b = -w * (np.cos(th) + np.sin(th)) / FL * SCALE
    return a.astype(np.float16), b.astype(np.float16)


def _window_correction():
    w = _window32()
    corr = np.zeros(OUT_LEN, dtype=np.float32)
    for j in range(4):
        chunk = w[j * FS:(j + 1) * FS]
        view = corr[j * FS:j * FS + NF * FS].reshape(NF, FS)
        view += chunk[None, :]
    return corr


def build_nc():
    nc = bacc.Bacc(trn_type="TRN2", target_bir_lowering=False, debug=False)
    sr_d = nc.dram_tensor("s_real", [FPC, FL], F16, kind="ExternalInput").ap()
    si_d = nc.dram_tensor("s_imag", [FPC, FL], F16, kind="ExternalInput").ap()
    ca_d = nc.dram_tensor("coef_a", [1, FL], F16, kind="ExternalInput").ap()
    cb_d = nc.dram_tensor("coef_b", [1, FL], F16, kind="ExternalInput").ap()
    id_d = nc.dram_tensor("ident", [P, P], F16, kind="ExternalInput").ap()
    out_d = nc.dram_tensor("out_seg", [P, OLEN], F16, kind="ExternalOutput").ap()

    sr3 = sr_d.rearrange("(p g) k -> p g k", p=P)
    si3 = si_d.rearrange("(p g) k -> p g k", p=P)

    with tile.TileContext(nc) as tc:
        with (
            tc.tile_pool(name="const", bufs=1) as cpool,
            tc.tile_pool(name="main", bufs=1) as mpool,
            tc.tile_pool(name="psum", bufs=1, space="PSUM") as ppool,
        ):
            At = cpool.tile([P, FL], F16, tag="At")
            Bt = cpool.tile([P, FL], F16, tag="Bt")
            It = cpool.tile([P, P], F16, tag="It")
            Srt = mpool.tile([P, G * FL], F16, tag="Sr")
            Sit = mpool.tile([P, G * FL], F16, tag="Si")
            P1t = mpool.tile([P, PFR * FL], F16, tag="P1")
            P2t = mpool.tile([P, PFR * FL], F16, tag="P2")
            # per-chunk output tiles and per-window PSUM tiles: distinct tags
            # so the tile framework doesn't serialize independent windows on
            # false whole-tile WAR hazards
            windows = [(0, 2), (2, 4), (4, 6), (6, 8), (8, 10), (10, 11)]
            Otc = [mpool.tile([P, n * FS], F16, tag=f"Oc{i}", name=f"Oc{i}")
                   for i, n in enumerate((4, 4, 3))]
            Opw = [ppool.tile([P, (b1 - b0) * FS], F32, tag=f"Ops{w}",
                              name=f"Ops{w}")
                   for w, (b0, b1) in enumerate(windows[:5])]

            # coefficient rows: tiny 2KB DMAs first in the sync queue, then
            # broadcast across partitions with K=1 ones-matmuls on the (idle
            # at startup) PE — keeps 0.5MB of broadcast traffic out of the
            # HBM stream
            Ar = cpool.tile([1, FL], F16, tag="Ar")
            Br = cpool.tile([1, FL], F16, tag="Br")
            Ones = cpool.tile([1, P], F16, tag="Ones")
            Cps = ppool.tile([P, FL], F32, tag="Cps")
            nc.sync.dma_start(out=Ar[:, :], in_=ca_d[:, :])
            nc.sync.dma_start(out=Br[:, :], in_=cb_d[:, :])
            nc.vector.memset(Ones[:, :], 1.0)
            for row, dst in ((Ar, At), (Br, Bt)):
                for h in range(2):
                    hs = slice(h * FL // 2, (h + 1) * FL // 2)
                    nc.tensor.matmul(Cps[:, hs], Ones[:, :], row[:, hs],
                                     start=True, stop=True)
                    nc.scalar.copy(out=dst[:, hs], in_=Cps[:, hs])
            nc.scalar.dma_start(out=It[:, :], in_=id_d[:, :])

            # zero the pad frames once on DVE (it is idle until the first
            # input quarter lands anyway)
            for T in (P1t, P2t):
                nc.vector.memset(T[:, 0:PAD * FL], 0.0)
                nc.vector.memset(T[:, (PAD + G) * FL:], 0.0)

            # input stream on the sync (SP) queue: 2-frame loads for frames
            # 0..5, single-frame loads for the last two so the product tail
            # after the final transfer is as short as possible
            loads = [(0, 2), (2, 2), (4, 2), (6, 1), (7, 1)]
            for f0, n in loads:
                sl = slice(f0 * FL, (f0 + n) * FL)
                nc.sync.dma_start(out=Srt[:, sl], in_=sr3[:, f0:f0 + n, :])
                nc.sync.dma_start(out=Sit[:, sl], in_=si3[:, f0:f0 + n, :])

            P1v = P1t[:, :].rearrange("p (g c j) -> p g c j", g=PFR, c=4)
            P2v = P2t[:, :].rearrange("p (g c j) -> p g c j", g=PFR, c=4)

            def emit_products(f0, n):
                sl = slice(f0 * FL, (f0 + n) * FL)
                psl = slice((PAD + f0) * FL, (PAD + f0 + n) * FL)
                for S, C, T in ((Srt, At, P1t), (Sit, Bt, P2t)):
                    nc.vector.tensor_tensor(
                        out=T[:, psl].rearrange("p (g k) -> p g k", g=n),
                        in0=S[:, sl].rearrange("p (g k) -> p g k", g=n),
                        in1=C[:, None, :].broadcast_to([P, n, FL]),
                        op=ALU.mult,
                    )

            def emit_window(w):
                # PE identity matmuls accumulate the chunk shifts x 2 tensors
                # into this window's private PSUM bank, then ACT copies to the
                # right output chunk as fp16.  Shifts whose whole frame range
                # falls in the pads are skipped (they would add zeros).
                b0, b1 = windows[w]
                Ow = Opw[w]
                Ov = Ow[:, :].rearrange("p (b j) -> p b j", b=b1 - b0)
                seq = [(t, c) for t in (0, 1) for c in range(4)
                       if (b1 - c > 0) and (b0 - c < G)]
                for i, (t, c) in enumerate(seq):
                    src = P1v if t == 0 else P2v
                    nc.tensor.matmul(
                        Ov[:, :, :], It[:, :],
                        src[:, b0 - c + PAD:b1 - c + PAD, c, :],
                        start=(i == 0), stop=(i == len(seq) - 1),
                    )
                chunk, coff = divmod(b0, 4)
                nc.scalar.copy(out=Otc[chunk][:, coff * FS:(coff + b1 - b0) * FS],
                               in_=Ow[:, :])

            def emit_window5_dve():
                # block 10 has exactly one real contribution pair (frame 7,
                # chunk 3): a single DVE add, running while PE drains windows
                # 3-4 — shortens the post-product tail
                nc.vector.tensor_tensor(
                    out=Otc[2][:, 2 * FS:3 * FS],
                    in0=P1v[:, G - 1 + PAD, 3, :],
                    in1=P2v[:, G - 1 + PAD, 3, :], op=ALU.add)

            # pipeline: products gate windows; emit in readiness order
            emit_products(0, 2)
            emit_products(2, 2)
            emit_window(0)          # frames <= 1
            emit_products(4, 2)
            emit_window(1)          # frames <= 3
            emit_products(6, 1)
            emit_window(2)          # frames <= 5
            nc.scalar.dma_start(out=out_d[:, 0:4 * FS], in_=Otc[0][:, :])
            emit_products(7, 1)
            emit_window(3)          # frames <= 7
            nc.scalar.dma_start(out=out_d[:, 4 * FS:8 * FS], in_=Otc[1][:, :])
            emit_window5_dve()
            emit_window(4)
            nc.scalar.dma_start(out=out_d[:, 8 * FS:], in_=Otc[2][:, :])
    nc.compile()
    return nc


_cache = {}


def _get_nc():
    if "nc" not in _cache:
        _cache["nc"] = build_nc()
    return _cache["nc"]


def make_in_maps(S_real, S_imag):
    a16, b16 = _coeffs16()
    sr16 = S_real.astype(np.float16)
    si16 = S_imag.astype(np.float16)
    ident = np.eye(P, dtype=np.float16)
    in_maps = []
    for m in range(NCORES):
        r0 = m * FPC
        in_maps.append({
            "s_real": np.ascontiguousarray(sr16[r0:r0 + FPC]),
            "s_imag": np.ascontiguousarray(si16[r0:r0 + FPC]),
            "coef_a": a16.reshape(1, FL),
            "coef_b": b16.reshape(1, FL),
            "ident": ident,
        })
    return in_maps


def assemble_output(segs):
    # segs: per core [128, 2816] fp16 padded OA partials
    acc = np.zeros((NF + NB - G, FS), dtype=np.float32)   # [8195, 256]
    for m in range(NCORES):
        seg = segs[m].astype(np.float32).reshape(P, NB, FS)
        accm = acc[FPC * m: FPC * m + FPC + NB - G]       # [1027, 256] view
        for b in range(NB):
            accm[b: b + (P - 1) * G + 1: G] += seg[:, b, :]
    if "corr" not in _cache:
        _cache["corr"] = _window_correction()
    corr = _cache["corr"]
    return (acc.reshape(-1) / (np.maximum(corr, np.float32(1e-8))
                               * np.float32(SCALE))).astype(np.float32)


def kernel(S_real, S_imag):
    S_real = np.asarray(S_real, dtype=np.float32)
    S_imag = np.asarray(S_imag, dtype=np.float32)
    in_maps = make_in_maps(S_real, S_imag)
    nc = _get_nc()
    res = run_bass_kernel_spmd(nc, in_maps, list(range(NCORES)))
    segs = [res.results[m]["out_seg"] for m in range(NCORES)]
    return assemble_output(segs)
